# revision 8
# baseline (speedup 1.0000x reference)
"""Trainium2 Bass kernel for nn_EquivarientScalar (segment_reduce).

Computation (reference): 2 stacked GatedEquivariant layers over N=100000
atoms (pointwise per atom), then sc = s @ out_w + out_b and a masked
segment-sum y[b] = sum_n sc[n] * batch_mask[b, n].

Strategy:
  - Data-parallel over atoms: 12500 atoms/core x 8 cores, padded to 12800
    (25 blocks of 512 atoms). Zero padding is exact (mask pad is zero).
  - Host pre-transposes to feature-major so every load is a clean
    contiguous DMA and all matmuls have the contraction dim on partitions:
      sT (128, 12800), vT (128, 3, 12800), maskT (12800, 128).
  - All matmuls in float32r (TF32-like, 1 cycle/row at N>=512).
  - Layer-2 vector output is discarded by the network, so layer 2 skips
    v1 and the gate, and a2w's scalar half is folded with out_w into a
    single (128,1) projection.
  - Segment reduce on-chip: sc columns via matmul with h2 chunks as the
    stationary operand, then y += maskT_chunk^T @ sc_col per 128 atoms.
  - Host sums the 8 per-core partial y vectors.
"""

import os
import sys

for _p in ("/opt/trn_rl_repo", "/root/.axon_site/_ro/trn_rl_repo"):
    if os.path.isdir(_p) and _p not in sys.path:
        sys.path.insert(0, _p)

import numpy as np

os.environ.setdefault("BASS_NEVER_TRACE", "1")  # no NTFF hook in this axon build

import concourse.bass as bass
import concourse.tile as tile
from concourse import bacc, mybir
from concourse.alu_op_type import AluOpType
from concourse.bass_utils import run_bass_kernel_spmd

N_CORES = 8
NA_FULL = 100000
NA_CORE = NA_FULL // N_CORES   # 12500
BLK = 512
NA = 12800                     # padded atoms per core (25 * 512)
NB = NA // BLK                 # 25 blocks
F = 128

F32 = mybir.dt.float32
F32R = mybir.dt.float32r
AF = mybir.ActivationFunctionType

W_NAMES = ["w1_0", "w2_0", "w2_1", "a1w_s0", "a1w_n0", "a1w_s1", "a1w_n1",
           "a2w_s0", "a2w_g0"]

_last_results = None  # BassKernelResults of the most recent run (for test harness)
_last_nc = None       # finalized Bass module of the most recent run
_last_in_maps = None  # per-core input dicts of the most recent run


def _build(b_fold: float):
    nc = bacc.Bacc("TRN2", target_bir_lowering=False)

    sT = nc.dram_tensor("sT", (F, NA), F32R, kind="ExternalInput")
    vT = nc.dram_tensor("vT", (F, 3, NA), F32R, kind="ExternalInput")
    mT = nc.dram_tensor("mT", (NA, F), F32R, kind="ExternalInput")
    wd = {n: nc.dram_tensor(n, (F, F), F32R, kind="ExternalInput") for n in W_NAMES}
    # (F, 2): col 0 = a2w_s1 @ out_w, col 1 = zeros — fp32r matmuls need an
    # even moving free dim, so the sc / y chains run at N=2 and use col 0.
    wf = nc.dram_tensor("w_fold", (F, 2), F32R, kind="ExternalInput")
    a1b0 = nc.dram_tensor("a1b0", (F, 1), F32, kind="ExternalInput")
    a1b1 = nc.dram_tensor("a1b1", (F, 1), F32, kind="ExternalInput")
    a2bs = nc.dram_tensor("a2bs", (F, 1), F32, kind="ExternalInput")
    a2bg = nc.dram_tensor("a2bg", (F, 1), F32, kind="ExternalInput")
    y = nc.dram_tensor("y", (F, 1), F32, kind="ExternalOutput")

    with tile.TileContext(nc) as tc:
        with tc.tile_pool(name="wpool", bufs=1) as wp, \
             tc.tile_pool(name="io", bufs=3) as io, \
             tc.tile_pool(name="work", bufs=2) as wk, \
             tc.tile_pool(name="psv", bufs=2, space="PSUM") as psv, \
             tc.tile_pool(name="psa", bufs=1, space="PSUM") as psa, \
             tc.tile_pool(name="pss", bufs=1, space="PSUM") as pss:

            wt = {}
            for n in W_NAMES:
                wt[n] = wp.tile([F, F], F32R, name=n, tag=n)
                nc.sync.dma_start(out=wt[n], in_=wd[n][:, :])
            wft = wp.tile([F, 2], F32R, tag="wf")
            nc.sync.dma_start(out=wft, in_=wf[:, :])
            bt = {}
            for n, d in [("a1b0", a1b0), ("a1b1", a1b1), ("a2bs", a2bs),
                         ("a2bg", a2bg)]:
                bt[n] = wp.tile([F, 1], F32, name=n, tag=n)
                nc.sync.dma_start(out=bt[n], in_=d[:, :])

            y_sb = wp.tile([F, 1], F32, tag="y_sb")
            nc.vector.memset(y_sb, 0.0)

            for b in range(NB):
                a0 = b * BLK

                sT_t = io.tile([F, BLK], F32R, tag="sT_t")
                nc.sync.dma_start(out=sT_t, in_=sT[:, a0:a0 + BLK])
                vT_t = io.tile([F, 3, BLK], F32R, tag="vT_t")
                nc.sync.dma_start(out=vT_t, in_=vT[:, :, a0:a0 + BLK])
                mT_t = io.tile([F, 4, F], F32R, tag="mT_t")
                nc.sync.dma_start(
                    out=mT_t,
                    in_=mT[a0:a0 + BLK, :].rearrange("(k p) g -> p k g", p=F),
                )

                # ---- layer 1 ----
                v2 = psv.tile([F, 3, BLK], F32, tag="vmm")
                for c in range(3):
                    nc.tensor.matmul(v2[:, c, :], wt["w2_0"], vT_t[:, c, :])
                sq = wk.tile([F, 3, BLK], F32, tag="sq")
                nc.scalar.activation(out=sq, in_=v2, func=AF.Square)
                tsum = wk.tile([F, BLK], F32, tag="tsum")
                nc.gpsimd.tensor_tensor(out=tsum, in0=sq[:, 0, :],
                                        in1=sq[:, 1, :], op=AluOpType.add)
                n2sq = wk.tile([F, BLK], F32, tag="n2sq")
                nc.gpsimd.tensor_tensor(out=n2sq, in0=tsum, in1=sq[:, 2, :],
                                        op=AluOpType.add)
                n2 = wk.tile([F, BLK], F32R, tag="n2")
                nc.scalar.activation(out=n2, in_=n2sq, func=AF.Sqrt)

                v1 = psv.tile([F, 3, BLK], F32, tag="vmm")
                for c in range(3):
                    nc.tensor.matmul(v1[:, c, :], wt["w1_0"], vT_t[:, c, :])

                a1 = psa.tile([F, BLK], F32, tag="a")
                nc.tensor.matmul(a1, wt["a1w_s0"], sT_t, start=True, stop=False)
                nc.tensor.matmul(a1, wt["a1w_n0"], n2, start=False, stop=True)
                h1 = wk.tile([F, BLK], F32R, tag="h1")
                nc.scalar.activation(out=h1, in_=a1, func=AF.Silu, bias=bt["a1b0"])

                a2g = psa.tile([F, BLK], F32, tag="a")
                nc.tensor.matmul(a2g, wt["a2w_g0"], h1)
                g = wk.tile([F, BLK], F32, tag="g")
                nc.vector.tensor_scalar(out=g, in0=a2g, scalar1=bt["a2bg"],
                                        scalar2=None, op0=AluOpType.add)

                a2s = psa.tile([F, BLK], F32, tag="a")
                nc.tensor.matmul(a2s, wt["a2w_s0"], h1)
                s1 = wk.tile([F, BLK], F32R, tag="s1")
                nc.vector.tensor_scalar(out=s1, in0=a2s, scalar1=bt["a2bs"],
                                        scalar2=None, op0=AluOpType.add)

                g_bc = bass.AP(tensor=g.tensor, offset=g.offset,
                               ap=[g.ap[0], [0, 3], g.ap[1]])
                vout = wk.tile([F, 3, BLK], F32R, tag="vout")
                nc.vector.tensor_tensor(out=vout, in0=v1, in1=g_bc,
                                        op=AluOpType.mult)

                # ---- layer 2 (scalar path only) ----
                v2b = psv.tile([F, 3, BLK], F32, tag="vmm")
                for c in range(3):
                    nc.tensor.matmul(v2b[:, c, :], wt["w2_1"], vout[:, c, :])
                sq2 = wk.tile([F, 3, BLK], F32, tag="sq")
                nc.scalar.activation(out=sq2, in_=v2b, func=AF.Square)
                tsum2 = wk.tile([F, BLK], F32, tag="tsum")
                nc.gpsimd.tensor_tensor(out=tsum2, in0=sq2[:, 0, :],
                                        in1=sq2[:, 1, :], op=AluOpType.add)
                n2sq2 = wk.tile([F, BLK], F32, tag="n2sq")
                nc.gpsimd.tensor_tensor(out=n2sq2, in0=tsum2, in1=sq2[:, 2, :],
                                        op=AluOpType.add)
                n2b = wk.tile([F, BLK], F32R, tag="n2")
                nc.scalar.activation(out=n2b, in_=n2sq2, func=AF.Sqrt)

                a1b_ = psa.tile([F, BLK], F32, tag="a")
                nc.tensor.matmul(a1b_, wt["a1w_s1"], s1, start=True, stop=False)
                nc.tensor.matmul(a1b_, wt["a1w_n1"], n2b, start=False, stop=True)
                h2 = wk.tile([F, BLK], F32R, tag="h2")
                nc.scalar.activation(out=h2, in_=a1b_, func=AF.Silu, bias=bt["a1b1"])

                # ---- sc and segment reduce ----
                for k in range(4):
                    sc_ps = pss.tile([F, 2], F32, tag="sc")
                    nc.tensor.matmul(sc_ps, h2[:, k * F:(k + 1) * F], wft)
                    sc_sb = wk.tile([F, 2], F32R, tag="sc_sb")
                    nc.vector.tensor_scalar(out=sc_sb, in0=sc_ps,
                                            scalar1=float(b_fold), scalar2=None,
                                            op0=AluOpType.add)
                    y_ps = pss.tile([F, 2], F32, tag="sc")
                    nc.tensor.matmul(y_ps, mT_t[:, k, :], sc_sb)
                    nc.vector.tensor_tensor(out=y_sb, in0=y_sb,
                                            in1=y_ps[:, 0:1],
                                            op=AluOpType.add)

            nc.sync.dma_start(out=y[:, :], in_=y_sb)

    nc.finalize()
    return nc


def kernel(s, v, r, batch_mask, w1, w2, a1w, a1b, a2w, a2b, out_w, out_b):
    global _last_results
    del r  # unused by the reference computation

    s = np.ascontiguousarray(np.asarray(s, dtype=np.float32)).reshape(NA_FULL, F)
    v = np.ascontiguousarray(np.asarray(v, dtype=np.float32)).reshape(NA_FULL, 3, F)
    batch_mask = np.ascontiguousarray(
        np.asarray(batch_mask, dtype=np.float32)).reshape(F, NA_FULL)
    w1 = np.asarray(w1, dtype=np.float32)
    w2 = np.asarray(w2, dtype=np.float32)
    a1w = np.asarray(a1w, dtype=np.float32)
    a1b = np.asarray(a1b, dtype=np.float32)
    a2w = np.asarray(a2w, dtype=np.float32)
    a2b = np.asarray(a2b, dtype=np.float32)
    out_w = np.asarray(out_w, dtype=np.float32)
    out_b = np.asarray(out_b, dtype=np.float32)
    assert w1.shape == (2, F, F), "kernel is specialized to L=2"

    # folded final projection: sc = h2 @ (a2w_s1 @ out_w) + (a2b_s1 @ out_w + out_b)
    w_fold = np.zeros((F, 2), dtype=np.float64)
    w_fold[:, 0:1] = (a2w[1][:, :F].astype(np.float64) @ out_w.astype(np.float64))
    b_fold = float(a2b[1][:F].astype(np.float64) @ out_w[:, 0].astype(np.float64)
                   + out_b[0])

    weights = {
        "w1_0": w1[0], "w2_0": w2[0], "w2_1": w2[1],
        "a1w_s0": a1w[0][:F, :], "a1w_n0": a1w[0][F:, :],
        "a1w_s1": a1w[1][:F, :], "a1w_n1": a1w[1][F:, :],
        "a2w_s0": a2w[0][:, :F], "a2w_g0": a2w[0][:, F:],
    }
    weights = {k: np.ascontiguousarray(a, dtype=np.float32)
               for k, a in weights.items()}
    w_fold32 = np.ascontiguousarray(w_fold, dtype=np.float32)
    bias_cols = {
        "a1b0": a1b[0].reshape(F, 1), "a1b1": a1b[1].reshape(F, 1),
        "a2bs": a2b[0][:F].reshape(F, 1), "a2bg": a2b[0][F:].reshape(F, 1),
    }
    bias_cols = {k: np.ascontiguousarray(a, dtype=np.float32)
                 for k, a in bias_cols.items()}

    in_maps = []
    for c in range(N_CORES):
        sl = slice(c * NA_CORE, (c + 1) * NA_CORE)
        sT = np.zeros((F, NA), dtype=np.float32)
        sT[:, :NA_CORE] = s[sl].T
        vT = np.zeros((F, 3, NA), dtype=np.float32)
        vT[:, :, :NA_CORE] = v[sl].transpose(2, 1, 0)
        mT = np.zeros((NA, F), dtype=np.float32)
        mT[:NA_CORE] = batch_mask[:, sl].T
        m = {"sT": sT, "vT": vT, "mT": mT, "w_fold": w_fold32}
        m.update(weights)
        m.update(bias_cols)
        in_maps.append(m)

    nc = _build(b_fold)
    res = run_bass_kernel_spmd(nc, in_maps, core_ids=list(range(N_CORES)))
    global _last_nc, _last_in_maps
    _last_results, _last_nc, _last_in_maps = res, nc, in_maps

    y = np.zeros((F, 1), dtype=np.float64)
    for c in range(N_CORES):
        y += res.results[c]["y"].astype(np.float64)
    return y.astype(np.float32)


# revision 14
# speedup vs baseline: 1.0872x; 1.0872x over previous
"""Trainium2 Bass kernel for nn_EquivarientScalar (segment_reduce).

Computation (reference): 2 stacked GatedEquivariant layers over N=100000
atoms (pointwise per atom), then sc = s @ out_w + out_b and a masked
segment-sum y[b] = sum_n sc[n] * batch_mask[b, n].

Strategy:
  - Data-parallel over atoms: 12500 atoms/core x 8 cores, padded to 12800
    (25 blocks of 512 atoms). Zero padding is exact (mask pad is zero).
  - Host pre-transposes to feature-major so every load is a clean
    contiguous DMA and all matmuls have the contraction dim on partitions:
      sT (128, 12800), vT (128, 3, 12800), maskT (12800, 128).
  - All matmuls in float32r (TF32-like, 1 cycle/row at N>=512).
  - Layer-2 vector output is discarded by the network, so layer 2 skips
    v1 and the gate, and a2w's scalar half is folded with out_w into a
    single (128,1) projection.
  - The vector-norm sqrt runs on GPSIMD+DVE (magic-constant rsqrt seed +
    two fitted Newton stages, max rel err ~7e-5), NOT on the ACT engine:
    ACT then only uses {Square, Silu, Copy}, which live in a single
    activation-table set, avoiding ~2.6us/block of ACT table reloads.
  - Segment reduce on-chip: sc columns via matmul with h2 chunks as the
    stationary operand, then y += maskT_chunk^T @ sc_col per 128 atoms.
  - Host sums the 8 per-core partial y vectors.
"""

import os
import sys

for _p in ("/opt/trn_rl_repo", "/root/.axon_site/_ro/trn_rl_repo"):
    if os.path.isdir(_p) and _p not in sys.path:
        sys.path.insert(0, _p)

os.environ.setdefault("BASS_NEVER_TRACE", "1")  # no NTFF hook in this axon build

import numpy as np

import concourse.bass as bass
import concourse.tile as tile
from concourse import bacc, mybir
from concourse import dve_ops as _dve_ops
from concourse.alu_op_type import AluOpType
from concourse.bass_utils import run_bass_kernel_spmd
from concourse.dve_ops import OPS as _DVE_OPS
from concourse.dve_ops import _CUSTOM_DVE_ROW_BASE, _SUB_OPCODE_FOR_NAME, DveOp
from concourse.dve_spec import C0 as _C0
from concourse.dve_spec import C1 as _C1
from concourse.dve_spec import Spec as _Spec
from concourse.dve_spec import Src0 as _Src0
from concourse.dve_spec import Src1 as _Src1
from concourse.dve_spec import lower as _dve_lower
from concourse.dve_uop import DveOpSpec as _DveOpSpec

N_CORES = 8
NA_FULL = 100000
NA_CORE = NA_FULL // N_CORES   # 12500
BLK = 512
NA = 12800                     # padded atoms per core (25 * 512)
NB = NA // BLK                 # 25 blocks
F = 128

F32 = mybir.dt.float32
F32R = mybir.dt.float32r
U32 = mybir.dt.uint32
AF = mybir.ActivationFunctionType

W_NAMES = ["w1_0", "w2_0", "w2_1", "a1w_s0", "a1w_n0", "a1w_s1", "a1w_n1",
           "a2w_s0", "a2w_g0"]

# ---- custom DVE sqrt: y0 = magic seed (computed on GPSIMD via dtype casts),
# then two fused Newton-ish stages on DVE. Constants fitted offline:
# wide-range fp32 max rel err ~7e-5; q=0 -> 0 (no NaN).
MAGIC_F = float(0x5F3759DF)
NR1_C0, NR1_C1 = 1.5005, 0.4996
FIN_C0, FIN_C1 = 1.49964329, 0.499642859

_last_results = None  # BassKernelResults of the most recent run (for test harness)
_last_nc = None       # finalized Bass module of the most recent run
_last_in_maps = None  # per-core input dicts of the most recent run


def _np_seed(q):
    f = q.view(np.uint32).astype(np.float64)
    bits = np.asarray(MAGIC_F - 0.5 * f, dtype=np.float32).astype(np.float64)
    return np.clip(bits, 0, 2**32 - 1).astype(np.uint32).view(np.float32)


def _ref_rsqrt_nr(in0, in1, c0, c1, c2):
    y0 = in1.astype(np.float32)
    return (y0 * (np.float32(c0) - np.float32(c1) * in0 * y0 * y0)).astype(
        np.float32)


def _ref_sqrt_fin(in0, in1, c0, c1, c2):
    qy = (in0 * in1).astype(np.float32)
    return (qy * (np.float32(c0) - np.float32(c1) * (qy * in1))).astype(np.float32)


def _register_ops():
    if "RSQRT_NR_ANT" in _SUB_OPCODE_FOR_NAME:
        by_name = {op.name: op for op in _DVE_OPS}
        return by_name["RSQRT_NR_ANT"], by_name["SQRT_FIN_ANT"]

    def make(name, body, ref):
        op = DveOp(name, _Spec(body=body, reference=ref), subdim=False,
                   uops_sha={})
        opcode = _CUSTOM_DVE_ROW_BASE + len(_DVE_OPS)
        for ver in ("v3", "v4"):
            try:
                spec = _DveOpSpec(name=name, opcode=opcode,
                                  uops=_dve_lower(op.spec, ver=ver),
                                  rd1_en=_dve_ops.has_src1(op.spec))
                op.uops_sha[ver] = spec.sha(ver)
            except Exception:
                pass
        _SUB_OPCODE_FOR_NAME[name] = opcode
        _DVE_OPS.append(op)
        return op

    # y1 = y0 * (C0 - C1 * q * y0^2)        [in0 = q, in1 = y0]
    nr1 = make("RSQRT_NR_ANT",
               _Src1 * (_C0 - _C1 * (_Src0 * (_Src1 * _Src1))),
               _ref_rsqrt_nr)
    # n2 = (q*y1) * (C0 - C1 * (q*y1*y1))   [in0 = q, in1 = y1]
    _qy = _Src0 * _Src1
    fin = make("SQRT_FIN_ANT",
               _qy * (_C0 - _C1 * (_qy * _Src1)),
               _ref_sqrt_fin)
    return nr1, fin


def _build(b_fold: float, reps: int = 1, trace_sim: bool = False):
    # reps > 1 repeats the whole computation inside one NEFF (timing only —
    # y then accumulates reps copies; used to subtract host/tunnel overhead).
    OP_NR1, OP_FIN = _register_ops()
    nc = bacc.Bacc("TRN2", target_bir_lowering=False)

    sT = nc.dram_tensor("sT", (F, NA), F32R, kind="ExternalInput")
    vT = nc.dram_tensor("vT", (F, 3, NA), F32R, kind="ExternalInput")
    mT = nc.dram_tensor("mT", (NA, F), F32R, kind="ExternalInput")
    wd = {n: nc.dram_tensor(n, (F, F), F32R, kind="ExternalInput") for n in W_NAMES}
    # (F, 2): col 0 = a2w_s1 @ out_w, col 1 = zeros — fp32r matmuls need an
    # even moving free dim, so the sc / y chains run at N=2 and use col 0.
    wf = nc.dram_tensor("w_fold", (F, 2), F32R, kind="ExternalInput")
    a1b0 = nc.dram_tensor("a1b0", (F, 1), F32, kind="ExternalInput")
    a1b1 = nc.dram_tensor("a1b1", (F, 1), F32, kind="ExternalInput")
    a2bs = nc.dram_tensor("a2bs", (F, 1), F32, kind="ExternalInput")
    a2bg = nc.dram_tensor("a2bg", (F, 1), F32, kind="ExternalInput")
    y = nc.dram_tensor("y", (F, 1), F32, kind="ExternalOutput")

    with tile.TileContext(nc, trace_sim=trace_sim) as tc:
        with tc.tile_pool(name="wpool", bufs=1) as wp, \
             tc.tile_pool(name="io", bufs=3) as io, \
             tc.tile_pool(name="work", bufs=2) as wk, \
             tc.tile_pool(name="psv", bufs=2, space="PSUM") as psv, \
             tc.tile_pool(name="psa", bufs=1, space="PSUM") as psa, \
             tc.tile_pool(name="pss", bufs=1, space="PSUM") as pss:

            wt = {}
            for n in W_NAMES:
                wt[n] = wp.tile([F, F], F32R, name=n, tag=n)
                nc.sync.dma_start(out=wt[n], in_=wd[n][:, :])
            wft = wp.tile([F, 2], F32R, tag="wf")
            nc.sync.dma_start(out=wft, in_=wf[:, :])
            bt = {}
            for n, d in [("a1b0", a1b0), ("a1b1", a1b1), ("a2bs", a2bs),
                         ("a2bg", a2bg)]:
                bt[n] = wp.tile([F, 1], F32, name=n, tag=n)
                nc.sync.dma_start(out=bt[n], in_=d[:, :])

            y_sb = wp.tile([F, 1], F32, tag="y_sb")
            nc.vector.memset(y_sb, 0.0)

            def norm_sqrt(n2sq, tag):
                # f = float(bits(q)); seedbits = u32(MAGIC - 0.5 f)  [GPSIMD]
                fb = wk.tile([F, BLK], F32, tag=f"fb_{tag}")
                nc.gpsimd.tensor_copy(out=fb, in_=n2sq.bitcast(U32))
                sd = wk.tile([F, BLK], F32, tag=f"sd_{tag}")
                nc.gpsimd.tensor_scalar(out=sd.bitcast(U32), in0=fb,
                                        scalar1=-0.5, scalar2=MAGIC_F,
                                        op0=AluOpType.mult, op1=AluOpType.add)
                # two Newton-ish stages [DVE]
                y1 = wk.tile([F, BLK], F32, tag=f"y1_{tag}")
                nc.vector._custom_dve(OP_NR1, out=y1, in0=n2sq, in1=sd,
                                      s0=NR1_C0, s1=NR1_C1)
                n2 = wk.tile([F, BLK], F32R, tag=f"n2_{tag}")
                nc.vector._custom_dve(OP_FIN, out=n2, in0=n2sq, in1=y1,
                                      s0=FIN_C0, s1=FIN_C1)
                return n2

            for b in range(NB * reps):
                a0 = (b % NB) * BLK

                sT_t = io.tile([F, BLK], F32R, tag="sT_t")
                nc.sync.dma_start(out=sT_t, in_=sT[:, a0:a0 + BLK])
                vT_t = io.tile([F, 3, BLK], F32R, tag="vT_t")
                nc.sync.dma_start(out=vT_t, in_=vT[:, :, a0:a0 + BLK])
                mT_t = io.tile([F, 4, F], F32R, tag="mT_t")
                nc.sync.dma_start(
                    out=mT_t,
                    in_=mT[a0:a0 + BLK, :].rearrange("(k p) g -> p k g", p=F),
                )

                # ---- layer 1 ----
                v2 = psv.tile([F, 3, BLK], F32, tag="vmm")
                for c in range(3):
                    nc.tensor.matmul(v2[:, c, :], wt["w2_0"], vT_t[:, c, :])
                sq = wk.tile([F, 3, BLK], F32, tag="sq")
                nc.scalar.activation(out=sq, in_=v2, func=AF.Square)
                tsum = wk.tile([F, BLK], F32, tag="tsum")
                nc.gpsimd.tensor_tensor(out=tsum, in0=sq[:, 0, :],
                                        in1=sq[:, 1, :], op=AluOpType.add)
                n2sq = wk.tile([F, BLK], F32, tag="n2sq")
                nc.gpsimd.tensor_tensor(out=n2sq, in0=tsum, in1=sq[:, 2, :],
                                        op=AluOpType.add)
                n2 = norm_sqrt(n2sq, "l1")

                v1 = psv.tile([F, 3, BLK], F32, tag="vmm")
                for c in range(3):
                    nc.tensor.matmul(v1[:, c, :], wt["w1_0"], vT_t[:, c, :])

                a1 = psa.tile([F, BLK], F32, tag="a")
                nc.tensor.matmul(a1, wt["a1w_s0"], sT_t, start=True, stop=False)
                nc.tensor.matmul(a1, wt["a1w_n0"], n2, start=False, stop=True)
                h1 = wk.tile([F, BLK], F32R, tag="h1")
                nc.scalar.activation(out=h1, in_=a1, func=AF.Silu, bias=bt["a1b0"])

                a2g = psa.tile([F, BLK], F32, tag="a")
                nc.tensor.matmul(a2g, wt["a2w_g0"], h1)
                g = wk.tile([F, BLK], F32, tag="g")
                nc.vector.tensor_scalar(out=g, in0=a2g, scalar1=bt["a2bg"],
                                        scalar2=None, op0=AluOpType.add)

                a2s = psa.tile([F, BLK], F32, tag="a")
                nc.tensor.matmul(a2s, wt["a2w_s0"], h1)
                s1 = wk.tile([F, BLK], F32R, tag="s1")
                nc.scalar.activation(out=s1, in_=a2s, func=AF.Identity,
                                     bias=bt["a2bs"])

                g_bc = bass.AP(tensor=g.tensor, offset=g.offset,
                               ap=[g.ap[0], [0, 3], g.ap[1]])
                vout = wk.tile([F, 3, BLK], F32R, tag="vout")
                nc.vector.tensor_tensor(out=vout, in0=v1, in1=g_bc,
                                        op=AluOpType.mult)

                # ---- layer 2 (scalar path only) ----
                v2b = psv.tile([F, 3, BLK], F32, tag="vmm")
                for c in range(3):
                    nc.tensor.matmul(v2b[:, c, :], wt["w2_1"], vout[:, c, :])
                sq2 = wk.tile([F, 3, BLK], F32, tag="sq")
                nc.scalar.activation(out=sq2, in_=v2b, func=AF.Square)
                tsum2 = wk.tile([F, BLK], F32, tag="tsum")
                nc.gpsimd.tensor_tensor(out=tsum2, in0=sq2[:, 0, :],
                                        in1=sq2[:, 1, :], op=AluOpType.add)
                n2sq2 = wk.tile([F, BLK], F32, tag="n2sq")
                nc.gpsimd.tensor_tensor(out=n2sq2, in0=tsum2, in1=sq2[:, 2, :],
                                        op=AluOpType.add)
                n2b = norm_sqrt(n2sq2, "l2")

                a1b_ = psa.tile([F, BLK], F32, tag="a")
                nc.tensor.matmul(a1b_, wt["a1w_s1"], s1, start=True, stop=False)
                nc.tensor.matmul(a1b_, wt["a1w_n1"], n2b, start=False, stop=True)
                h2 = wk.tile([F, BLK], F32R, tag="h2")
                nc.scalar.activation(out=h2, in_=a1b_, func=AF.Silu, bias=bt["a1b1"])

                # ---- sc and segment reduce ----
                for k in range(4):
                    sc_ps = pss.tile([F, 2], F32, tag="sc")
                    nc.tensor.matmul(sc_ps, h2[:, k * F:(k + 1) * F], wft)
                    sc_sb = wk.tile([F, 2], F32R, tag="sc_sb")
                    nc.vector.tensor_scalar(out=sc_sb, in0=sc_ps,
                                            scalar1=float(b_fold), scalar2=None,
                                            op0=AluOpType.add)
                    y_ps = pss.tile([F, 2], F32, tag="sc")
                    nc.tensor.matmul(y_ps, mT_t[:, k, :], sc_sb)
                    nc.vector.tensor_tensor(out=y_sb, in0=y_sb,
                                            in1=y_ps[:, 0:1],
                                            op=AluOpType.add)

            nc.sync.dma_start(out=y[:, :], in_=y_sb)

    nc.finalize()
    return nc


def kernel(s, v, r, batch_mask, w1, w2, a1w, a1b, a2w, a2b, out_w, out_b):
    global _last_results
    del r  # unused by the reference computation

    s = np.ascontiguousarray(np.asarray(s, dtype=np.float32)).reshape(NA_FULL, F)
    v = np.ascontiguousarray(np.asarray(v, dtype=np.float32)).reshape(NA_FULL, 3, F)
    batch_mask = np.ascontiguousarray(
        np.asarray(batch_mask, dtype=np.float32)).reshape(F, NA_FULL)
    w1 = np.asarray(w1, dtype=np.float32)
    w2 = np.asarray(w2, dtype=np.float32)
    a1w = np.asarray(a1w, dtype=np.float32)
    a1b = np.asarray(a1b, dtype=np.float32)
    a2w = np.asarray(a2w, dtype=np.float32)
    a2b = np.asarray(a2b, dtype=np.float32)
    out_w = np.asarray(out_w, dtype=np.float32)
    out_b = np.asarray(out_b, dtype=np.float32)
    assert w1.shape == (2, F, F), "kernel is specialized to L=2"

    # folded final projection: sc = h2 @ (a2w_s1 @ out_w) + (a2b_s1 @ out_w + out_b)
    w_fold = np.zeros((F, 2), dtype=np.float64)
    w_fold[:, 0:1] = (a2w[1][:, :F].astype(np.float64) @ out_w.astype(np.float64))
    b_fold = float(a2b[1][:F].astype(np.float64) @ out_w[:, 0].astype(np.float64)
                   + out_b[0])

    weights = {
        "w1_0": w1[0], "w2_0": w2[0], "w2_1": w2[1],
        "a1w_s0": a1w[0][:F, :], "a1w_n0": a1w[0][F:, :],
        "a1w_s1": a1w[1][:F, :], "a1w_n1": a1w[1][F:, :],
        "a2w_s0": a2w[0][:, :F], "a2w_g0": a2w[0][:, F:],
    }
    weights = {k: np.ascontiguousarray(a, dtype=np.float32)
               for k, a in weights.items()}
    w_fold32 = np.ascontiguousarray(w_fold, dtype=np.float32)
    bias_cols = {
        "a1b0": a1b[0].reshape(F, 1), "a1b1": a1b[1].reshape(F, 1),
        "a2bs": a2b[0][:F].reshape(F, 1), "a2bg": a2b[0][F:].reshape(F, 1),
    }
    bias_cols = {k: np.ascontiguousarray(a, dtype=np.float32)
                 for k, a in bias_cols.items()}

    in_maps = []
    for c in range(N_CORES):
        sl = slice(c * NA_CORE, (c + 1) * NA_CORE)
        sT = np.zeros((F, NA), dtype=np.float32)
        sT[:, :NA_CORE] = s[sl].T
        vT = np.zeros((F, 3, NA), dtype=np.float32)
        vT[:, :, :NA_CORE] = v[sl].transpose(2, 1, 0)
        mT = np.zeros((NA, F), dtype=np.float32)
        mT[:NA_CORE] = batch_mask[:, sl].T
        m = {"sT": sT, "vT": vT, "mT": mT, "w_fold": w_fold32}
        m.update(weights)
        m.update(bias_cols)
        in_maps.append(m)

    nc = _build(b_fold)
    res = run_bass_kernel_spmd(nc, in_maps, core_ids=list(range(N_CORES)))
    global _last_nc, _last_in_maps
    _last_results, _last_nc, _last_in_maps = res, nc, in_maps

    y = np.zeros((F, 1), dtype=np.float64)
    for c in range(N_CORES):
        y += res.results[c]["y"].astype(np.float64)
    return y.astype(np.float32)


# revision 29
# speedup vs baseline: 16.1500x; 14.8549x over previous
"""Trainium2 Bass kernel for nn_EquivarientScalar (segment_reduce).

Computation (reference): 2 stacked GatedEquivariant layers over N=100000
atoms (pointwise per atom), then sc = s @ out_w + out_b and a masked
segment-sum y[b] = sum_n sc[n] * batch_mask[b, n].

Strategy:
  - Data-parallel over atoms: 12500 atoms/core x 8 cores, padded to 12800
    (25 blocks of 512 atoms). Zero padding is exact (mask pad is zero).
  - Host pre-transposes to feature-major so every load is a clean
    contiguous DMA and all matmuls have the contraction dim on partitions:
      sT (128, 12800), vT (128, 3, 12800), maskT (12800, 128).
  - All matmuls in float32r (TF32-like, 1 cycle/row at N>=512).
  - Layer-2 vector output is discarded by the network, so layer 2 skips
    v1 and the gate, and a2w's scalar half is folded with out_w into a
    single (128,1) projection.
  - The vector-norm sqrt runs on GPSIMD+DVE (magic-constant rsqrt seed +
    two fitted Newton stages, max rel err ~7e-5), NOT on the ACT engine:
    ACT then only uses {Square, Silu, Copy}, which live in a single
    activation-table set, avoiding ~2.6us/block of ACT table reloads.
  - Segment reduce on-chip: sc columns via matmul with h2 chunks as the
    stationary operand, then y += maskT_chunk^T @ sc_col per 128 atoms.
  - Host sums the 8 per-core partial y vectors.
"""

import os
import sys

for _p in ("/opt/trn_rl_repo", "/root/.axon_site/_ro/trn_rl_repo"):
    if os.path.isdir(_p) and _p not in sys.path:
        sys.path.insert(0, _p)

os.environ.setdefault("BASS_NEVER_TRACE", "1")  # no NTFF hook in this axon build

import numpy as np

import concourse.bass as bass
import concourse.tile as tile
from concourse import bacc, mybir
from concourse import dve_ops as _dve_ops
from concourse.alu_op_type import AluOpType
from concourse.bass_utils import run_bass_kernel_spmd
from concourse.dve_ops import OPS as _DVE_OPS
from concourse.dve_ops import _CUSTOM_DVE_ROW_BASE, _SUB_OPCODE_FOR_NAME, DveOp
from concourse.dve_spec import C0 as _C0
from concourse.dve_spec import C1 as _C1
from concourse.dve_spec import Spec as _Spec
from concourse.dve_spec import Src0 as _Src0
from concourse.dve_spec import Src1 as _Src1
from concourse.dve_spec import lower as _dve_lower
from concourse.dve_uop import DveOpSpec as _DveOpSpec

N_CORES = 8
NA_FULL = 100000
NA_CORE = NA_FULL // N_CORES   # 12500
BLK = 512
NA = 12800                     # padded atoms per core (25 * 512)
NB = NA // BLK                 # 25 blocks
F = 128

F32 = mybir.dt.float32
F32R = mybir.dt.float32r
U32 = mybir.dt.uint32
AF = mybir.ActivationFunctionType

W_NAMES = ["w1_0", "w2_0", "w2_1", "a1w_s0", "a1w_n0", "a1w_s1", "a1w_n1",
           "a2w_s0", "a2w_g0"]

# ---- custom DVE sqrt: y0 = magic seed (computed on GPSIMD via dtype casts),
# then two fused Newton-ish stages on DVE. Constants fitted offline:
# wide-range fp32 max rel err ~7e-5; q=0 -> 0 (no NaN).
MAGIC_F = 1596013007.0          # tuned jointly with the polynomial below
SQ1_C0, SQ1_C1 = 1.6695484, 0.688087555  # n2 = (q*y0)*(C0 - C1*q*y0^2)

_last_results = None  # BassKernelResults of the most recent run (for test harness)
_last_nc = None       # finalized Bass module of the most recent run
_last_in_maps = None  # per-core input dicts of the most recent run


def _np_seed(q):
    f = q.view(np.uint32).astype(np.float64)
    bits = np.asarray(MAGIC_F - 0.5 * f, dtype=np.float32).astype(np.float64)
    return np.clip(bits, 0, 2**32 - 1).astype(np.uint32).view(np.float32)


def _ref_rsqrt_nr(in0, in1, c0, c1, c2):
    y0 = in1.astype(np.float32)
    return (y0 * (np.float32(c0) - np.float32(c1) * in0 * y0 * y0)).astype(
        np.float32)


def _ref_sqrt_fin(in0, in1, c0, c1, c2):
    qy = (in0 * in1).astype(np.float32)
    return (qy * (np.float32(c0) - np.float32(c1) * (qy * in1))).astype(np.float32)


def _register_ops():
    if "RSQRT_NR_ANT" in _SUB_OPCODE_FOR_NAME:
        by_name = {op.name: op for op in _DVE_OPS}
        return by_name["RSQRT_NR_ANT"], by_name["SQRT_FIN_ANT"]

    def make(name, body, ref):
        op = DveOp(name, _Spec(body=body, reference=ref), subdim=False,
                   uops_sha={})
        opcode = _CUSTOM_DVE_ROW_BASE + len(_DVE_OPS)
        for ver in ("v3", "v4"):
            try:
                spec = _DveOpSpec(name=name, opcode=opcode,
                                  uops=_dve_lower(op.spec, ver=ver),
                                  rd1_en=_dve_ops.has_src1(op.spec))
                op.uops_sha[ver] = spec.sha(ver)
            except Exception:
                pass
        _SUB_OPCODE_FOR_NAME[name] = opcode
        _DVE_OPS.append(op)
        return op

    # y1 = y0 * (C0 - C1 * q * y0^2)        [in0 = q, in1 = y0]
    nr1 = make("RSQRT_NR_ANT",
               _Src1 * (_C0 - _C1 * (_Src0 * (_Src1 * _Src1))),
               _ref_rsqrt_nr)
    # n2 = (q*y1) * (C0 - C1 * (q*y1*y1))   [in0 = q, in1 = y1]
    _qy = _Src0 * _Src1
    fin = make("SQRT_FIN_ANT",
               _qy * (_C0 - _C1 * (_qy * _Src1)),
               _ref_sqrt_fin)
    return nr1, fin


def _build(b_fold: float, reps: int = 1, trace_sim: bool = False):
    # reps > 1 repeats the whole computation inside one NEFF (timing only —
    # y then accumulates reps copies; used to subtract host/tunnel overhead).
    OP_NR1, OP_FIN = _register_ops()
    nc = bacc.Bacc("TRN2", target_bir_lowering=False)

    # packed per-block input: per partition row = [sT 512 | vT 3*512 | mask 4*128]
    xp = nc.dram_tensor("xpack", (NB, F, 5 * BLK), F32R, kind="ExternalInput")
    wd = {n: nc.dram_tensor(n, (F, F), F32R, kind="ExternalInput") for n in W_NAMES}
    # (F, 2): col 0 = a2w_s1 @ out_w, col 1 = zeros — fp32r matmuls need an
    # even moving free dim, so the sc / y chains run at N=2 and use col 0.
    wf = nc.dram_tensor("w_fold", (F, 2), F32R, kind="ExternalInput")
    a1b0 = nc.dram_tensor("a1b0", (F, 1), F32, kind="ExternalInput")
    a1b1 = nc.dram_tensor("a1b1", (F, 1), F32, kind="ExternalInput")
    a2bs = nc.dram_tensor("a2bs", (F, 1), F32, kind="ExternalInput")
    a2bg = nc.dram_tensor("a2bg", (F, 1), F32, kind="ExternalInput")
    y = nc.dram_tensor("y", (F, 1), F32, kind="ExternalOutput")

    with tile.TileContext(nc, trace_sim=trace_sim) as tc:
        # PSUM budget (8 banks): v01 tag 2 banks x2 bufs + vc2 tag 1 bank x2
        # bufs + "a" tag (a1/a2g/a2s/a1_L2/sc/y) 1 bank x2 bufs = 8.
        with tc.tile_pool(name="wpool", bufs=1) as wp, \
             tc.tile_pool(name="io", bufs=5) as io, \
             tc.tile_pool(name="work", bufs=4) as wk, \
             tc.tile_pool(name="psv", bufs=2, space="PSUM") as psv, \
             tc.tile_pool(name="psa", bufs=2, space="PSUM") as psa:

            wt = {}
            for n in W_NAMES:
                wt[n] = wp.tile([F, F], F32R, name=n, tag=n)
                nc.sync.dma_start(out=wt[n], in_=wd[n][:, :])
            wft = wp.tile([F, 2], F32R, tag="wf")
            nc.sync.dma_start(out=wft, in_=wf[:, :])
            bt = {}
            for n, d in [("a1b0", a1b0), ("a1b1", a1b1), ("a2bs", a2bs),
                         ("a2bg", a2bg)]:
                bt[n] = wp.tile([F, 1], F32, name=n, tag=n)
                nc.sync.dma_start(out=bt[n], in_=d[:, :])

            y_sb = wp.tile([F, 1], F32, tag="y_sb")
            nc.vector.memset(y_sb, 0.0)

            def norm_sqrt(n2sq, tag):
                # seedbits = u32(MAGIC - 0.5 * float(bits(q))) in ONE GPSIMD op:
                # the u32 input AP value-casts to fp32 on read, the u32 output
                # AP value-casts back on write.
                sd = wk.tile([F, BLK], F32, tag=f"sd_{tag}")
                nc.gpsimd.tensor_scalar(out=sd.bitcast(U32),
                                        in0=n2sq.bitcast(U32),
                                        scalar1=-0.5, scalar2=MAGIC_F,
                                        op0=AluOpType.mult, op1=AluOpType.add)
                # single fused Newton stage [DVE]: n2 = (q*y0)*(C0-C1*q*y0^2)
                n2 = wk.tile([F, BLK], F32R, tag=f"n2_{tag}")
                nc.vector._custom_dve(OP_FIN, out=n2, in0=n2sq, in1=sd,
                                      s0=SQ1_C0, s1=SQ1_C1)
                return n2

            for b in range(NB * reps):
                xt = io.tile([F, 5 * BLK], F32R, tag="xt")
                with tc.high_priority(offset=110):
                    nc.sync.dma_start(out=xt, in_=xp[b % NB])
                sT_t = xt[:, 0:BLK]
                vT_t = xt[:, BLK:4 * BLK].rearrange("p (c a) -> p c a", c=3)
                mT_t = xt[:, 4 * BLK:].rearrange("p (k g) -> p k g", k=BLK // F)

                def norm_front(w2t, rhs01, rhs2, tag):
                    # v2 matmuls split 2+1 over spatial c for finer PSUM
                    # recycling; squares on ACT; spatial sum on GPSIMD.
                    v01 = psv.tile([F, 2, BLK], F32, tag="v01")
                    nc.tensor.matmul(v01[:, 0, :], w2t, rhs01[0])
                    nc.tensor.matmul(v01[:, 1, :], w2t, rhs01[1])
                    sq01 = wk.tile([F, 2, BLK], F32, tag="sq01")
                    nc.scalar.activation(out=sq01, in_=v01, func=AF.Square)
                    vc2 = psv.tile([F, BLK], F32, tag="vc2")
                    nc.tensor.matmul(vc2, w2t, rhs2)
                    sqc2 = wk.tile([F, BLK], F32, tag="sqc2")
                    nc.scalar.activation(out=sqc2, in_=vc2, func=AF.Square)
                    tsum = wk.tile([F, BLK], F32, tag="tsum")
                    nc.gpsimd.tensor_tensor(out=tsum, in0=sq01[:, 0, :],
                                            in1=sq01[:, 1, :], op=AluOpType.add)
                    n2sq = wk.tile([F, BLK], F32, tag="n2sq")
                    nc.gpsimd.tensor_tensor(out=n2sq, in0=tsum, in1=sqc2,
                                            op=AluOpType.add)
                    return norm_sqrt(n2sq, tag)

                # ---- layer 1 ----
                n2 = norm_front(wt["w2_0"], (vT_t[:, 0, :], vT_t[:, 1, :]),
                                vT_t[:, 2, :], "l1")

                a1 = psa.tile([F, BLK], F32, tag="a")
                nc.tensor.matmul(a1, wt["a1w_s0"], sT_t, start=True, stop=False)
                nc.tensor.matmul(a1, wt["a1w_n0"], n2, start=False, stop=True)
                h1 = wk.tile([F, BLK], F32R, tag="h1")
                nc.scalar.activation(out=h1, in_=a1, func=AF.Silu, bias=bt["a1b0"])

                a2g = psa.tile([F, BLK], F32, tag="a")
                nc.tensor.matmul(a2g, wt["a2w_g0"], h1)
                g = wk.tile([F, BLK], F32, tag="g")
                nc.vector.tensor_scalar(out=g, in0=a2g, scalar1=bt["a2bg"],
                                        scalar2=None, op0=AluOpType.add)

                a2s = psa.tile([F, BLK], F32, tag="a")
                nc.tensor.matmul(a2s, wt["a2w_s0"], h1)
                s1 = wk.tile([F, BLK], F32R, tag="s1")
                nc.vector.tensor_scalar(out=s1, in0=a2s, scalar1=bt["a2bs"],
                                        scalar2=None, op0=AluOpType.add)

                # v1 matmuls emitted late (they're only needed by the gate):
                # this keeps the shared v01/vc2 PSUM slots free through the
                # first half of the block so the next block's front can start.
                v101 = psv.tile([F, 2, BLK], F32, tag="v01")
                nc.tensor.matmul(v101[:, 0, :], wt["w1_0"], vT_t[:, 0, :])
                nc.tensor.matmul(v101[:, 1, :], wt["w1_0"], vT_t[:, 1, :])
                v1c2 = psv.tile([F, BLK], F32, tag="vc2")
                nc.tensor.matmul(v1c2, wt["w1_0"], vT_t[:, 2, :])

                g_bc2 = bass.AP(tensor=g.tensor, offset=g.offset,
                                ap=[g.ap[0], [0, 2], g.ap[1]])
                vout = wk.tile([F, 3, BLK], F32R, tag="vout")
                nc.vector.tensor_tensor(out=vout[:, 0:2, :], in0=v101,
                                        in1=g_bc2, op=AluOpType.mult)
                nc.vector.tensor_tensor(out=vout[:, 2, :], in0=v1c2, in1=g,
                                        op=AluOpType.mult)

                # ---- layer 2 (scalar path only) ----
                n2b = norm_front(wt["w2_1"], (vout[:, 0, :], vout[:, 1, :]),
                                 vout[:, 2, :], "l2")

                a1b_ = psa.tile([F, BLK], F32, tag="a")
                nc.tensor.matmul(a1b_, wt["a1w_s1"], s1, start=True, stop=False)
                nc.tensor.matmul(a1b_, wt["a1w_n1"], n2b, start=False, stop=True)
                h2 = wk.tile([F, BLK], F32R, tag="h2")
                nc.scalar.activation(out=h2, in_=a1b_, func=AF.Silu, bias=bt["a1b1"])

                # ---- sc and segment reduce (batched through one PSUM bank) ----
                sc_ps = psa.tile([F, BLK // F, 2], F32, tag="a")
                for k in range(BLK // F):
                    nc.tensor.matmul(sc_ps[:, k, :], h2[:, k * F:(k + 1) * F],
                                     wft)
                sc_sb = wk.tile([F, BLK // F, 2], F32R, tag="sc_sb")
                nc.vector.tensor_scalar(out=sc_sb, in0=sc_ps,
                                        scalar1=float(b_fold), scalar2=None,
                                        op0=AluOpType.add)
                y_ps = psa.tile([F, 2], F32, tag="a")
                for k in range(BLK // F):
                    nc.tensor.matmul(y_ps, mT_t[:, k, :], sc_sb[:, k, :],
                                     start=(k == 0), stop=(k == BLK // F - 1),
                                     skip_group_check=True)
                nc.vector.tensor_tensor(out=y_sb, in0=y_sb, in1=y_ps[:, 0:1],
                                        op=AluOpType.add)

            nc.sync.dma_start(out=y[:, :], in_=y_sb)

    nc.finalize()
    return nc


def kernel(s, v, r, batch_mask, w1, w2, a1w, a1b, a2w, a2b, out_w, out_b):
    global _last_results
    del r  # unused by the reference computation

    s = np.ascontiguousarray(np.asarray(s, dtype=np.float32)).reshape(NA_FULL, F)
    v = np.ascontiguousarray(np.asarray(v, dtype=np.float32)).reshape(NA_FULL, 3, F)
    batch_mask = np.ascontiguousarray(
        np.asarray(batch_mask, dtype=np.float32)).reshape(F, NA_FULL)
    w1 = np.asarray(w1, dtype=np.float32)
    w2 = np.asarray(w2, dtype=np.float32)
    a1w = np.asarray(a1w, dtype=np.float32)
    a1b = np.asarray(a1b, dtype=np.float32)
    a2w = np.asarray(a2w, dtype=np.float32)
    a2b = np.asarray(a2b, dtype=np.float32)
    out_w = np.asarray(out_w, dtype=np.float32)
    out_b = np.asarray(out_b, dtype=np.float32)
    assert w1.shape == (2, F, F), "kernel is specialized to L=2"

    # folded final projection: sc = h2 @ (a2w_s1 @ out_w) + (a2b_s1 @ out_w + out_b)
    w_fold = np.zeros((F, 2), dtype=np.float64)
    w_fold[:, 0:1] = (a2w[1][:, :F].astype(np.float64) @ out_w.astype(np.float64))
    b_fold = float(a2b[1][:F].astype(np.float64) @ out_w[:, 0].astype(np.float64)
                   + out_b[0])

    weights = {
        "w1_0": w1[0], "w2_0": w2[0], "w2_1": w2[1],
        "a1w_s0": a1w[0][:F, :], "a1w_n0": a1w[0][F:, :],
        "a1w_s1": a1w[1][:F, :], "a1w_n1": a1w[1][F:, :],
        "a2w_s0": a2w[0][:, :F], "a2w_g0": a2w[0][:, F:],
    }
    weights = {k: np.ascontiguousarray(a, dtype=np.float32)
               for k, a in weights.items()}
    w_fold32 = np.ascontiguousarray(w_fold, dtype=np.float32)
    bias_cols = {
        "a1b0": a1b[0].reshape(F, 1), "a1b1": a1b[1].reshape(F, 1),
        "a2bs": a2b[0][:F].reshape(F, 1), "a2bg": a2b[0][F:].reshape(F, 1),
    }
    bias_cols = {k: np.ascontiguousarray(a, dtype=np.float32)
                 for k, a in bias_cols.items()}

    in_maps = []
    for c in range(N_CORES):
        sl = slice(c * NA_CORE, (c + 1) * NA_CORE)
        sT = np.zeros((F, NA), dtype=np.float32)
        sT[:, :NA_CORE] = s[sl].T
        vT = np.zeros((F, 3, NA), dtype=np.float32)
        vT[:, :, :NA_CORE] = v[sl].transpose(2, 1, 0)
        mT = np.zeros((NA, F), dtype=np.float32)
        mT[:NA_CORE] = batch_mask[:, sl].T
        xp = np.empty((NB, F, 5 * BLK), dtype=np.float32)
        xp[:, :, 0:BLK] = sT.reshape(F, NB, BLK).transpose(1, 0, 2)
        xp[:, :, BLK:4 * BLK] = (
            vT.reshape(F, 3, NB, BLK).transpose(2, 0, 1, 3).reshape(NB, F, 3 * BLK))
        xp[:, :, 4 * BLK:] = (
            mT.reshape(NB, BLK // F, F, F).transpose(0, 2, 1, 3)
            .reshape(NB, F, BLK))
        m = {"xpack": xp, "w_fold": w_fold32}
        m.update(weights)
        m.update(bias_cols)
        in_maps.append(m)

    nc = _build(b_fold)
    res = run_bass_kernel_spmd(nc, in_maps, core_ids=list(range(N_CORES)))
    global _last_nc, _last_in_maps
    _last_results, _last_nc, _last_in_maps = res, nc, in_maps

    y = np.zeros((F, 1), dtype=np.float64)
    for c in range(N_CORES):
        y += res.results[c]["y"].astype(np.float64)
    return y.astype(np.float32)
